# revision 1
# baseline (speedup 1.0000x reference)
"""Trainium2 Bass kernel for nn_DE3 (histogram_binning + entropy).

Full input: img [16, 2048, 2048] f32 with values in [0, 256).
reference = B * (8 - res), res = -sum p log2 p, p = bincount(floor(img)) / (H*W).

Strategy (8 NeuronCores, data parallel):
  - Split the 64Mi elements into 8 shards of 8Mi (one per core).
  - Per core, compute the 2-D cumulative-count matrix
        J[i, j] = #{e : hi_e >= i AND lo_e >= j}, i in [0,NHI), j in [0,NLO)
    where idx = floor(x) = NLO*hi + lo (NHI x NLO = 256 bins). J is
    accumulated on the PE (one [128,NHI]x[128,NLO] bf16 matmul per 128
    elements into a single PSUM tile; NHI=8 keeps the LDWEIGHTS stream
    short). The hi/lo "ladders" (is_ge cumulants) are built on DVE/ACT
    at a few cycles per element via the float32 round-to-int trick.
  - Host: sum J over cores, 2-D finite difference -> 256-bin counts,
    then the trivial entropy epilogue.
"""

import numpy as np

import concourse.bass as bass
import concourse.mybir as mybir
from concourse.tile import TileContext
from concourse.bass_utils import run_bass_kernel_spmd

P = 128          # SBUF partitions
F = 512          # free-dim elements per tile
N_CORES = 8
NHI = 16         # hi (coarse) bins  -> matmul M / LDWEIGHTS columns
NLO = 16         # lo (fine) bins within a block -> matmul N
assert NHI * NLO == 256

_BIG = float(3 * 2**22)  # 1.5*2^23: keeps t in [2^23, 2^24) where ulp = 1

_MAX_WAITS = 1  # this walrus build supports at most 1 sync-wait per instruction


def _split_excess_waits(nc):
    """Walrus in this container rejects instructions with >2 sync-wait
    commands (Tile's tail drain can carry many). Move excess waits onto
    same-engine NoOp instructions inserted just before the offender."""
    n_split = 0
    for f in nc.m.functions:
        for bb in f.blocks:
            out = []
            for ins in bb.instructions:
                si = getattr(ins, "sync_info", None)
                waits = list(si.on_wait) if si is not None and si.on_wait else []
                if len(waits) > _MAX_WAITS:
                    extra, keep = waits[:-_MAX_WAITS], waits[-_MAX_WAITS:]
                    for ci in range(0, len(extra), _MAX_WAITS):
                        chunk = extra[ci : ci + _MAX_WAITS]
                        nop = mybir.InstNoOp(
                            name=f"{ins.name}-wsplit{ci}",
                            engine=ins.engine,
                            sync_info=mybir.SyncInfo(on_wait=chunk, on_update=[]),
                        )
                        out.append(nop)
                        n_split += 1
                    si.on_wait = keep
                out.append(ins)
            bb.instructions = out
    return n_split


def build_nc(n_tiles: int, debug: bool = False, repeat: int = 1, col_tiles: int = 1):
    """Build the Bass kernel: input [n_tiles*P, F] f32 -> output J [16,16] f32."""
    nc = bass.Bass()
    # const AP for the ACT-engine bias (-2^23), mirroring Bass's own init
    _ct = nc.alloc_sbuf_tensor("const-neg-big", [128, 1], mybir.dt.float32)
    nc.gpsimd.memset(_ct.ap(), -_BIG)
    nc.const_aps.aps[(mybir.dt.float32, -_BIG)] = _ct.ap()
    nc.all_engine_barrier()
    x_in = nc.declare_dram_parameter(
        "x", [n_tiles * P, F], mybir.dt.float32, isOutput=False
    )
    j_out = nc.declare_dram_parameter(
        "j", [col_tiles * NHI, NLO], mybir.dt.float32, isOutput=True
    )
    if debug:
        dbg_hi = nc.declare_dram_parameter("dbg_hi", [P, F], mybir.dt.float32, isOutput=True)
        dbg_lo = nc.declare_dram_parameter("dbg_lo", [P, F], mybir.dt.float32, isOutput=True)
        dbg_lhi = nc.declare_dram_parameter("dbg_lhi", [P, NHI * F], mybir.dt.float32, isOutput=True)
        dbg_llo = nc.declare_dram_parameter("dbg_llo", [P, NLO * F], mybir.dt.float32, isOutput=True)

    dt = mybir.dt
    op = mybir.AluOpType

    with TileContext(nc) as tc:
        with (
            tc.tile_pool(name="data", bufs=3) as dpool,
            tc.tile_pool(name="lad", bufs=2) as lpool,
            tc.tile_pool(name="psum", bufs=1, space="PSUM") as ppool,
            tc.tile_pool(name="outp", bufs=1) as opool,
        ):
            # col_tiles > 1: spread chunks round-robin over PE column
            # groups; each group accumulates its own J slice at PSUM
            # partition base 32*g (summed on the host afterwards).
            jt = ppool.tile([32 * (col_tiles - 1) + NHI, NLO], dt.float32)
            for rep in range(repeat):
              for it in range(n_tiles):
                  x = dpool.tile([P, F], dt.float32, tag="x")
                  nc.sync.dma_start(out=x[:], in_=x_in[it * P : (it + 1) * P, :])
                  # xb = x - NLO/2 (exact); carries the -0.5 through /NLO for
                  # the floor-by-round trick (BIG-0.5 is not representable).
                  xb = dpool.tile([P, F], dt.float32, tag="xb")
                  nc.vector.tensor_scalar(
                      out=xb[:], in0=x[:], scalar1=-float(NLO) / 2.0, scalar2=None, op0=op.add
                  )
                  # t = xb/NLO + BIG = (x/NLO - 0.5) + BIG -> RN: BIG + floor(x/NLO)
                  t = dpool.tile([P, F], dt.float32, tag="t")
                  nc.vector.tensor_scalar(
                      out=t[:], in0=xb[:], scalar1=1.0 / float(NLO), scalar2=_BIG,
                      op0=op.mult, op1=op.add,
                  )
                  # hi = t - BIG in [0,NHI], exact small int -> bf16 (ACT engine)
                  hi8 = dpool.tile([P, F], dt.bfloat16, tag="hi8")
                  nc.scalar.add(hi8[:], t[:], -_BIG)
                  # yb = xb - NLO*hi = (x - NLO*hi) - NLO/2  in [-NLO/2, NLO/2)
                  yb = dpool.tile([P, F], dt.float32, tag="yb")
                  nc.vector.scalar_tensor_tensor(
                      out=yb[:], in0=hi8[:], scalar=-float(NLO), in1=xb[:],
                      op0=op.mult, op1=op.add,
                  )
                  # u = (yb + (NLO/2 - 0.5)) + BIG -> RN: BIG + lo
                  u = dpool.tile([P, F], dt.float32, tag="u")
                  nc.vector.tensor_scalar(
                      out=u[:], in0=yb[:], scalar1=float(NLO) / 2.0 - 0.5, scalar2=_BIG,
                      op0=op.add, op1=op.add,
                  )
                  # lo = u - BIG in [0,NLO], exact small int -> bf16 (ACT engine)
                  lo8 = dpool.tile([P, F], dt.bfloat16, tag="lo8")
                  nc.scalar.add(lo8[:], u[:], -_BIG)

                  # ladders: lhi[p, i, f] = (hi >= i), llo[p, j, f] = (lo >= j)
                  lhi = lpool.tile([P, NHI, F], dt.bfloat16, tag="lhi")
                  llo = lpool.tile([P, NLO, F], dt.bfloat16, tag="llo")
                  for j in range(NHI):
                      nc.vector.tensor_scalar(
                          out=lhi[:, j, :], in0=hi8[:], scalar1=float(j), scalar2=None,
                          op0=op.is_ge,
                      )
                  for j in range(NLO):
                      nc.vector.tensor_scalar(
                          out=llo[:, j, :], in0=lo8[:], scalar1=float(j), scalar2=None,
                          op0=op.is_ge,
                      )
                  if debug and it == 0:
                      fhi = dpool.tile([P, F], dt.float32, tag="fhi")
                      nc.vector.tensor_copy(out=fhi[:], in_=hi8[:])
                      nc.sync.dma_start(out=dbg_hi[:], in_=fhi[:])
                      flo = dpool.tile([P, F], dt.float32, tag="flo")
                      nc.vector.tensor_copy(out=flo[:], in_=lo8[:])
                      nc.sync.dma_start(out=dbg_lo[:], in_=flo[:])
                      flh = lpool.tile([P, NHI * F], dt.float32, tag="flh")
                      nc.vector.tensor_copy(out=flh[:], in_=lhi[:].rearrange('p a b -> p (a b)'))
                      nc.sync.dma_start(out=dbg_lhi[:], in_=flh[:])
                      fll = lpool.tile([P, NLO * F], dt.float32, tag="fll")
                      nc.vector.tensor_copy(out=fll[:], in_=llo[:].rearrange('p a b -> p (a b)'))
                      nc.sync.dma_start(out=dbg_llo[:], in_=fll[:])
                  # PE: accumulate J += lhi_c^T @ llo_c for each 128-elem column c
                  for c in range(F):
                      g = c % col_tiles
                      nc.tensor.matmul(
                          jt[32 * g : 32 * g + NHI, :],
                          lhsT=lhi[:, :, c],
                          rhs=llo[:, :, c],
                          start=(rep == 0 and it == 0 and c < col_tiles),
                          stop=(rep == repeat - 1 and it == n_tiles - 1 and c >= F - col_tiles),
                          tile_position=(0, 32 * g) if col_tiles > 1 else None,
                      )
            jsb = opool.tile([32 * (col_tiles - 1) + NHI, NLO], dt.float32)
            for g in range(col_tiles):
                nc.vector.tensor_copy(
                    out=jsb[32 * g : 32 * g + NHI, :],
                    in_=jt[32 * g : 32 * g + NHI, :],
                )
                nc.sync.dma_start(
                    out=j_out[g * NHI : (g + 1) * NHI, :],
                    in_=jsb[32 * g : 32 * g + NHI, :],
                )
    _split_excess_waits(nc)
    return nc


def _counts_from_J(J: np.ndarray) -> np.ndarray:
    """J [NHI,NLO] cumulative -> counts [256] (bin = NLO*hi + lo)."""
    Jp = np.zeros((NHI + 1, NLO + 1), dtype=np.float64)
    Jp[:NHI, :NLO] = J
    A = Jp[:NHI, :] - Jp[1:, :]
    c2 = A[:, :NLO] - A[:, 1:]
    return c2.reshape(256)


def kernel(img: np.ndarray) -> np.ndarray:
    img = np.asarray(img, dtype=np.float32)
    B, H, W = img.shape
    flat = img.reshape(-1)
    n = flat.size
    assert n % (N_CORES * P * F) == 0
    shard = n // N_CORES
    n_tiles = shard // (P * F)

    nc = build_nc(n_tiles)
    in_maps = [
        {"x": flat[i * shard : (i + 1) * shard].reshape(n_tiles * P, F)}
        for i in range(N_CORES)
    ]
    res = run_bass_kernel_spmd(nc, in_maps, list(range(N_CORES)))
    J = np.zeros((NHI, NLO), dtype=np.float64)
    for r in res.results:
        J += np.asarray(r["j"], dtype=np.float64)

    counts = _counts_from_J(J)
    temp = float(H * W)
    p = counts / temp
    with np.errstate(divide="ignore", invalid="ignore"):
        terms = np.where(p > 0, p * np.log2(np.where(p > 0, p, 1.0)), 0.0)
    ent = -terms.sum()
    out = np.float32(B * (8.0 - ent))
    return np.asarray(out, dtype=np.float32)

